# revision 33
# baseline (speedup 1.0000x reference)
"""MultiHeadEMABlock Trainium2 kernel (8-core SPMD, bass/Tile).

Math (reference):
  h = LayerNorm_c(x[b,c,n] over c) * gamma + beta          (per (b,n))
  xe[b,n,h,d] = h[b,n,d] * expansion[h,d]
  y = causal damped EMA along n: y[t] = a_h*sum_{s<=t} q_h^{t-s} xe[s]
  out[b,d,n] = sum_h y[b,n,h,d]*reduction[h,d] + x

Identities used:
  - Per-(h,d) scales commute with the EMA (it mixes along n only):
      out = x + sum_h rho_h[d] * S_h[d,n],  rho_h[d] = a_h*e[h,d]*r[h,d]*gamma[d]
      S_h = EMA(q_h) applied to the normalized input z.
  - beta contributes a data-independent low-rank term added on host (exact).

Sharding: 8 cores = 4 batches x 2 sequence halves. Each core processes its
half plus a W-column halo from the left (zero-padded for the first half);
q^W underflows, so results are exact without any cross-core collective.

Device algorithm (per core, c-major [channel x n] base layout):
  1. LayerNorm stats via replicated ones-matmuls on PE; z = (x-m)*rstd (DVE),
     rstd = exp(-0.5*ln(var+eps)) on ACT (Rsqrt table is unusable here).
  2. EMA as chunked causal convolution on PE, chunk L=128:
     - scale+transpose fused: one matmul per (chunk,dtile,headgroup) with a
       diag(rho_h) packed rhs turns c-major z into n-major per-head scaled
       inputs X_h (4 heads per N=512 matmul).
     - per chunk, 8 lower-triangular T_h matmuls head-accumulate in PSUM,
       plus a K=8 rank-8 carry-correction matmul (q_h^{i+1} profiles).
     - carries tracked per head via an unscaled transpose + end-row matmul
       (E), propagated with tiny [8,512] DVE ops.
  3. Back-transpose to c-major via identity matmuls, residual add on GpSimd,
     DMA out.
"""
import contextlib
import ctypes
import sys
import types

import numpy as np

for _p in ("/root/.axon_site/_ro/trn_rl_repo", "/opt/trn_rl_repo"):
    if _p not in sys.path:
        sys.path.append(_p)

B, C, N, H = 4, 512, 4096, 8
EPS = 1e-5
N_CORES = 8
NHALF = N // 2
CT = C // 128  # channel tiles
L = 128  # EMA chunk length


# ---------------------------------------------------------------------------
# axon NTFF shim (lets run_bass_kernel_spmd(trace=True) capture HW profiles)
# ---------------------------------------------------------------------------
def _install_ntff_shim():
    if "antenv.axon_hooks" in sys.modules:
        return
    holder = {"hook": None}

    def _make(so_path):
        try:
            lib = ctypes.CDLL(so_path)
        except OSError:
            return None
        if not hasattr(lib, "axon_start_nrt_profile"):
            return None
        lib.axon_start_nrt_profile.argtypes = [
            ctypes.POINTER(ctypes.c_int64),
            ctypes.c_size_t,
        ]
        lib.axon_start_nrt_profile.restype = ctypes.c_int64
        lib.axon_stop_nrt_profile.argtypes = [ctypes.c_char_p]
        lib.axon_stop_nrt_profile.restype = ctypes.c_int64

        @contextlib.contextmanager
        def _hook(output_dir, device_ids):
            import jax

            jax.devices()
            if device_ids:
                ids = (ctypes.c_int64 * len(device_ids))(*device_ids)
                rc = lib.axon_start_nrt_profile(ids, len(device_ids))
            else:
                rc = lib.axon_start_nrt_profile(None, 0)
            if rc != 0:
                raise RuntimeError(f"axon_start_nrt_profile rc={rc}")
            try:
                yield
            finally:
                n = lib.axon_stop_nrt_profile(str(output_dir).encode())
                print(f"ntff profile: {n} file(s) -> {output_dir}", file=sys.stderr)

        return _hook

    mod = types.ModuleType("antenv.axon_hooks")
    mod.set_axon_ntff_profile_hook = lambda h: holder.__setitem__("hook", h)
    mod.get_axon_ntff_profile_hook = lambda: holder["hook"]
    sys.modules["antenv.axon_hooks"] = mod
    try:
        import antenv

        antenv.axon_hooks = mod
    except ImportError:
        pass
    holder["hook"] = _make("/opt/axon/libaxon_pjrt.so")


def _split_multiwait(nc, max_waits=1):
    """This walrus build rejects >1 sync wait per instruction; split extras
    onto same-engine NoOps inserted just before (per-engine order is the
    execution order, so semantics are preserved)."""
    from concourse import mybir

    k = [0]
    for fn in nc.m.functions:
        for blk in fn.blocks:
            out = []
            for inst in blk.instructions:
                si = getattr(inst, "sync_info", None)
                if si is not None and len(si.on_wait) > max_waits:
                    waits = list(si.on_wait)
                    for w in waits[max_waits:]:
                        k[0] += 1
                        out.append(
                            mybir.InstNoOp(
                                name=f"{inst.name}-mw{k[0]}",
                                sync_info=mybir.SyncInfo(on_wait=[w], on_update=[]),
                                bass_nofuse=True,
                                engine=inst.engine,
                            )
                        )
                    inst.sync_info = mybir.SyncInfo(
                        on_wait=waits[:max_waits], on_update=list(si.on_update)
                    )
                out.append(inst)
            blk.instructions[:] = out


# ---------------------------------------------------------------------------
# program builder
# ---------------------------------------------------------------------------
def build_program(W):
    """Build the SPMD per-core program. W: halo width (multiple of L)."""
    import concourse.bass as bass
    import concourse.tile as tile
    from concourse import mybir

    NW = NHALF + W
    K0 = W // L
    NCH = NW // L  # chunks
    # ragged 512-wide stat chunks
    stat_slices = []
    o = 0
    while o < NW:
        w = min(512, NW - o)
        stat_slices.append((o, w))
        o += w
    f32 = mybir.dt.float32
    bf16 = mybir.dt.bfloat16
    Op = mybir.AluOpType
    Act = mybir.ActivationFunctionType

    nc = bass.Bass(
        "TRN2",
        target_bir_lowering=False,
        debug=False,
        enable_asserts=False,
        num_devices=N_CORES,
    )
    xs_d = nc.dram_tensor("xs", [C, NW], f32, kind="ExternalInput").ap()
    tm_d = nc.dram_tensor("tmats", [H * 128, 128], bf16, kind="ExternalInput").ap()
    w4_d = nc.dram_tensor("w4", [H * 128, 512], bf16, kind="ExternalInput").ap()
    ek_d = nc.dram_tensor("ek", [128, H], bf16, kind="ExternalInput").ap()
    pm_d = nc.dram_tensor("pmat", [H, 128], bf16, kind="ExternalInput").ap()
    id_d = nc.dram_tensor("ident", [128, 128], bf16, kind="ExternalInput").ap()
    rh_d = nc.dram_tensor("rho_hd", [H, C], f32, kind="ExternalInput").ap()
    ql_d = nc.dram_tensor("qlcol", [H, 1], f32, kind="ExternalInput").ap()
    out_d = nc.dram_tensor("out_t", [C, NHALF], f32, kind="ExternalOutput").ap()

    with tile.TileContext(nc) as tc:
        with contextlib.ExitStack() as ctx:
            pers = ctx.enter_context(tc.tile_pool(name="pers", bufs=1))
            xs_pool = ctx.enter_context(tc.tile_pool(name="xsp", bufs=2))
            sq_pool = ctx.enter_context(tc.tile_pool(name="sqp", bufs=4))
            ps_pool = ctx.enter_context(tc.tile_pool(name="ps", bufs=1, space="PSUM"))
            st_pool = ctx.enter_context(tc.tile_pool(name="stats", bufs=3))
            xh_pool = ctx.enter_context(tc.tile_pool(name="xhp", bufs=4))
            xu_pool = ctx.enter_context(tc.tile_pool(name="xup", bufs=4))
            cr_pool = ctx.enter_context(tc.tile_pool(name="crp", bufs=3))
            s_pool = ctx.enter_context(tc.tile_pool(name="sp", bufs=4))
            out_pool = ctx.enter_context(tc.tile_pool(name="outp", bufs=4))

            # ---- small constants (sync queue, cheap) ----
            ek = pers.tile([128, H], bf16, tag="ek")
            nc.sync.dma_start(out=ek[:], in_=ek_d)
            pmat = pers.tile([H, 128], bf16, tag="pmat")
            nc.sync.dma_start(out=pmat[:], in_=pm_d)
            ident = pers.tile([128, 128], bf16, tag="ident")
            nc.sync.dma_start(out=ident[:], in_=id_d)
            rho = pers.tile([H, C], f32, tag="rho")
            nc.sync.dma_start(out=rho[:], in_=rh_d)
            qlc = pers.tile([H, 1], f32, tag="qlc")
            nc.sync.dma_start(out=qlc[:], in_=ql_d)
            epsb = pers.tile([128, 1], f32, tag="eps")
            nc.gpsimd.memset(epsb[:], EPS)
            ones = pers.tile([128, 128], bf16, tag="ones")
            nc.gpsimd.memset(ones[:], 1.0 / C)
            # big constants on the scalar HWDGE queue so they don't delay xs
            T8 = [pers.tile([128, 128], bf16, tag=f"T{h}", name=f"T{h}") for h in range(H)]
            for h in range(H):
                nc.scalar.dma_start(out=T8[h][:], in_=tm_d[h * 128 : (h + 1) * 128, :])
            W4 = [pers.tile([128, 512], bf16, tag=f"W4_{i}", name=f"W4_{i}") for i in range(H)]
            for i in range(H):
                nc.scalar.dma_start(out=W4[i][:], in_=w4_d[i * 128 : (i + 1) * 128, :])

            # ---- load, cast, square (per stat-chunk pieces for fast ramp) ----
            xb = pers.tile([128, CT * NW], bf16, tag="xb")
            z = pers.tile([128, CT * NW], bf16, tag="z")
            xsq = [pers.tile([128, NW], bf16, tag=f"sq{ct}", name=f"sq{ct}")
                   for ct in range(CT)]
            for o, wd in stat_slices:
                for ct in range(CT):
                    xst = xs_pool.tile([128, 512], f32, tag="xs", bufs=6)
                    nc.sync.dma_start(
                        out=xst[:, :wd],
                        in_=xs_d[ct * 128 : (ct + 1) * 128, o : o + wd],
                    )
                    nc.vector.tensor_scalar(
                        out=xb[:, ct * NW + o : ct * NW + o + wd], in0=xst[:, :wd],
                        scalar1=1.0, scalar2=None, op0=Op.mult,
                    )
                    nc.scalar.square(out=xsq[ct][:, o : o + wd], in_=xst[:, :wd])

            # ---- layernorm stats + z ----
            for o, wd in stat_slices:
                ps_m = ps_pool.tile([128, 512], f32, tag="ema", bufs=2)
                ps_s = ps_pool.tile([128, 512], f32, tag="ema", bufs=2)
                for ct in range(CT):
                    nc.tensor.matmul(
                        out=ps_m[:, :wd], lhsT=ones[:],
                        rhs=xb[:, ct * NW + o : ct * NW + o + wd],
                        start=(ct == 0), stop=(ct == CT - 1),
                    )
                for ct in range(CT):
                    nc.tensor.matmul(
                        out=ps_s[:, :wd], lhsT=ones[:], rhs=xsq[ct][:, o : o + wd],
                        start=(ct == 0), stop=(ct == CT - 1),
                    )
                mean_bf = st_pool.tile([128, 512], bf16, tag="meanbf")
                nc.scalar.activation(out=mean_bf[:, :wd], in_=ps_m[:, :wd], func=Act.Copy)
                m2 = st_pool.tile([128, 512], f32, tag="m2")
                nc.scalar.square(out=m2[:, :wd], in_=ps_m[:, :wd])
                var = st_pool.tile([128, 512], f32, tag="var")
                nc.vector.scalar_tensor_tensor(
                    out=var[:, :wd], in0=ps_s[:, :wd], scalar=0.0, in1=m2[:, :wd],
                    op0=Op.bypass, op1=Op.subtract,
                )
                lnv = st_pool.tile([128, 512], f32, tag="lnv")
                nc.scalar.activation(out=lnv[:, :wd], in_=var[:, :wd], func=Act.Ln, bias=epsb[:])
                rstd = st_pool.tile([128, 512], bf16, tag="rstd")
                nc.scalar.activation(out=rstd[:, :wd], in_=lnv[:, :wd], func=Act.Exp, scale=-0.5)
                for ct in range(CT):
                    t = st_pool.tile([128, 512], bf16, tag="tnorm")
                    nc.vector.tensor_tensor(
                        out=t[:, :wd], in0=xb[:, ct * NW + o : ct * NW + o + wd],
                        in1=mean_bf[:, :wd], op=Op.subtract,
                    )
                    nc.vector.tensor_tensor(
                        out=z[:, ct * NW + o : ct * NW + o + wd], in0=t[:, :wd],
                        in1=rstd[:, :wd], op=Op.mult,
                    )

            # ---- EMA chunks ----
            c_cur = cr_pool.tile([H, C], f32, tag="carry")
            nc.gpsimd.memset(c_cur[:], 0.0)

            def z_slice(k, dt):
                return z[:, dt * NW + k * L : dt * NW + (k + 1) * L]

            def carry_end(k):
                """X_u transpose + end-row matmul E_k; returns e_ps."""
                xu_ps = ps_pool.tile([128, 512], f32, tag="misc", bufs=2)
                for dt in range(CT):
                    nc.tensor.matmul(
                        out=xu_ps[:, dt * 128 : (dt + 1) * 128],
                        lhsT=z_slice(k, dt), rhs=ident[:], start=True, stop=True,
                    )
                xu = xu_pool.tile([128, 512], bf16, tag="xu")
                nc.scalar.activation(out=xu[:], in_=xu_ps[:], func=Act.Copy)
                e_ps = ps_pool.tile([H, 512], f32, tag="misc", bufs=2)
                nc.tensor.matmul(out=e_ps[:], lhsT=ek[:], rhs=xu[:], start=True,
                                 stop=True)
                return e_ps

            def carry_update(c_prev, e_ps):
                c_nxt = cr_pool.tile([H, C], f32, tag="carry")
                c_tmp = cr_pool.tile([H, C], f32, tag="ctmp")
                nc.vector.tensor_scalar(
                    out=c_tmp[:], in0=c_prev[:], scalar1=qlc[:, 0:1], scalar2=None,
                    op0=Op.mult,
                )
                nc.vector.tensor_tensor(out=c_nxt[:], in0=c_tmp[:], in1=e_ps[:],
                                        op=Op.add)
                return c_nxt

            def make_xh(k):
                """scaled transposes: xh cols = g*2048 + dt*512 + h'*128 + jj"""
                xh = xh_pool.tile([128, H * 512], bf16, tag="xh")
                for g in range(2):
                    for dp in range(2):
                        sp = ps_pool.tile([128, 1024], f32, tag="xps", bufs=2)
                        for dd in range(2):
                            dt = dp * 2 + dd
                            nc.tensor.matmul(
                                out=sp[:, dd * 512 : (dd + 1) * 512],
                                lhsT=z_slice(k, dt), rhs=W4[g * CT + dt][:],
                                start=True, stop=True,
                            )
                        dst = xh[:, g * 2048 + dp * 1024 : g * 2048 + (dp + 1) * 1024]
                        if (g + dp) % 2 == 0:
                            nc.scalar.activation(out=dst, in_=sp[:], func=Act.Copy)
                        else:
                            nc.vector.tensor_scalar(
                                out=dst, in0=sp[:], scalar1=1.0, scalar2=None,
                                op0=Op.mult,
                            )
                return xh

            def make_crho(c):
                c_rho = cr_pool.tile([H, C], bf16, tag="crho")
                nc.vector.tensor_tensor(out=c_rho[:], in0=c[:], in1=rho[:], op=Op.mult)
                return c_rho

            def chunk_tail(k, bps):
                """bps is already channel-major [d, i]: residual + store"""
                ot = out_pool.tile([128, 512], f32, tag="out")
                resid = xb.rearrange("p (dt t) -> p dt t", dt=CT)[
                    :, :, k * L : (k + 1) * L
                ]
                nc.vector.tensor_tensor(
                    out=ot[:].rearrange("p (dt i) -> p dt i", dt=CT),
                    in0=bps[:].rearrange("p (dt i) -> p dt i", dt=CT),
                    in1=resid, op=Op.add,
                )
                ko = k - K0
                nc.sync.dma_start(
                    out=out_d.rearrange("(dt p) n -> p dt n", dt=CT)[
                        :, :, ko * L : (ko + 1) * L
                    ],
                    in_=ot[:].rearrange("p (dt i) -> p dt i", dt=CT),
                )

            for k in range(K0):  # halo chunks: carries only
                e_ps = carry_end(k)
                c_cur = carry_update(c_cur, e_ps)

            ks = list(range(K0, NCH))
            pairs = [ks[i : i + 2] for i in range(0, len(ks), 2)]
            for pair in pairs:
                xhs, crhos, psums = [], [], []
                for k in pair:
                    last = k == NCH - 1
                    e_ps = None if last else carry_end(k)
                    xhs.append(make_xh(k))
                    crhos.append(make_crho(c_cur))
                    if not last:
                        c_cur = carry_update(c_cur, e_ps)
                # transposed-output EMA: out[d, i] = sum_j X_h[j, d] * T_h[i, j]
                # (stationary = xh 128-col slices, moving = T8[h]); result lands
                # channel-major in PSUM, no back-transpose needed.
                for i, k in enumerate(pair):
                    bps = ps_pool.tile([128, 512], f32, tag="ema", bufs=2,
                                       name=f"emaps{k}")
                    psums.append(bps)
                    for dt in range(CT):
                        for h in range(H):
                            g, hp = divmod(h, 4)
                            col = g * 2048 + dt * 512 + hp * 128
                            nc.tensor.matmul(
                                out=bps[:, dt * 128 : (dt + 1) * 128],
                                lhsT=xhs[i][:, col : col + 128], rhs=T8[h][:],
                                start=(h == 0), stop=False,
                            )
                        nc.tensor.matmul(
                            out=bps[:, dt * 128 : (dt + 1) * 128],
                            lhsT=crhos[i][:, dt * 128 : (dt + 1) * 128],
                            rhs=pmat[:], start=False, stop=True,
                        )
                for i, k in enumerate(pair):
                    chunk_tail(k, psums[i])
    return nc


def _host_params(ln_gamma, ln_beta, expansion, reduction, alphas, dampen_factors):
    import ml_dtypes

    a = 1.0 / (1.0 + np.exp(-alphas.astype(np.float64)))
    q = (1.0 - a) / (1.0 + np.exp(-dampen_factors.astype(np.float64)))
    qmax = float(q.max())
    W = L
    while qmax**W > 1e-12 and W < NHALF:
        W += L
    rho = (
        a[:, None]
        * expansion.astype(np.float64)
        * reduction.astype(np.float64)
        * ln_gamma.astype(np.float64)[None, :]
    )  # [H, C]
    bf = ml_dtypes.bfloat16
    ii, jj = np.meshgrid(np.arange(L), np.arange(L), indexing="ij")
    tmats = np.zeros((H * 128, 128), bf)
    for h in range(H):
        M = np.where(ii >= jj, q[h] ** np.maximum(ii - jj, 0), 0.0)  # T_h[i,j]
        tmats[h * 128 : (h + 1) * 128, :] = M.T.astype(bf)  # lhsT[j,i]
    w4 = np.zeros((H * 128, 512), bf)
    for g in range(2):
        for dt in range(CT):
            blk = np.zeros((128, 512))
            for hp in range(4):
                h = g * 4 + hp
                blk[:, hp * 128 : (hp + 1) * 128] = np.diag(rho[h, dt * 128 : (dt + 1) * 128])
            w4[(g * CT + dt) * 128 : (g * CT + dt + 1) * 128, :] = blk.astype(bf)
    ek = np.zeros((128, H), bf)
    for h in range(H):
        ek[:, h] = (q[h] ** (L - 1 - np.arange(L))).astype(bf)
    pmat = np.zeros((H, 128), bf)
    for h in range(H):
        pmat[h, :] = (q[h] ** (np.arange(L) + 1.0)).astype(bf)
    ident = np.eye(128, dtype=bf)
    rho_hd = rho.astype(np.float32)
    qlcol = (q**L).astype(np.float32).reshape(H, 1)
    consts = dict(
        tmats=tmats, w4=w4, ek=ek, pmat=pmat, ident=ident, rho_hd=rho_hd,
        qlcol=qlcol,
    )
    return a, q, W, consts


def _beta_term(ln_beta, expansion, reduction, a, q):
    if not np.any(ln_beta):
        return None
    n_idx = np.arange(N, dtype=np.float64)
    Cn = a[:, None] * (1.0 - q[:, None] ** (n_idx[None, :] + 1.0)) / (1.0 - q[:, None])
    w = (
        expansion.astype(np.float64)
        * reduction.astype(np.float64)
        * ln_beta.astype(np.float64)[None, :]
    )
    return np.einsum("hc,hn->cn", w, Cn).astype(np.float32)


def _make_in_maps(x, W, consts):
    NW = NHALF + W
    in_maps = []
    for core in range(N_CORES):
        b, half = divmod(core, 2)
        xs = np.zeros((C, NW), np.float32)
        s = half * NHALF - W
        if s < 0:
            xs[:, W:] = x[b, :, :NHALF]
        else:
            xs[:] = x[b, :, s : s + NW]
        in_maps.append(dict(consts, xs=xs))
    return in_maps


def kernel(x, ln_gamma, ln_beta, expansion, reduction, alphas, dampen_factors,
           trace=False):
    _install_ntff_shim()
    from concourse.bass_utils import run_bass_kernel_spmd
    from concourse.bass_interp import get_hw_module

    x = np.asarray(x, np.float32)
    a, q, W, consts = _host_params(
        np.asarray(ln_gamma), np.asarray(ln_beta), np.asarray(expansion),
        np.asarray(reduction), np.asarray(alphas), np.asarray(dampen_factors),
    )
    nc = build_program(W)
    _split_multiwait(nc)
    nc.m = get_hw_module(nc.m)

    in_maps = _make_in_maps(x, W, consts)
    res = run_bass_kernel_spmd(
        nc, in_maps, core_ids=list(range(N_CORES)), trace=trace
    )

    out = np.empty((B, C, N), np.float32)
    for core in range(N_CORES):
        b, half = divmod(core, 2)
        out[b, :, half * NHALF : (half + 1) * NHALF] = res.results[core]["out_t"]
    bt = _beta_term(
        np.asarray(ln_beta), np.asarray(expansion), np.asarray(reduction), a, q
    )
    if bt is not None:
        out += bt[None]
    if trace:
        kernel.last_results = res
    return out


# revision 34
# speedup vs baseline: 1.0535x; 1.0535x over previous
"""MultiHeadEMABlock Trainium2 kernel (8-core SPMD, bass/Tile).

Math (reference):
  h = LayerNorm_c(x[b,c,n] over c) * gamma + beta          (per (b,n))
  xe[b,n,h,d] = h[b,n,d] * expansion[h,d]
  y = causal damped EMA along n: y[t] = a_h*sum_{s<=t} q_h^{t-s} xe[s]
  out[b,d,n] = sum_h y[b,n,h,d]*reduction[h,d] + x

Identities used:
  - Per-(h,d) scales commute with the EMA (it mixes along n only):
      out = x + sum_h rho_h[d] * S_h[d,n],  rho_h[d] = a_h*e[h,d]*r[h,d]*gamma[d]
      S_h = EMA(q_h) applied to the normalized input z.
  - beta contributes a data-independent low-rank term added on host (exact).

Sharding: 8 cores = 4 batches x 2 sequence halves. Each core processes its
half plus a W-column halo from the left (zero-padded for the first half);
q^W underflows, so results are exact without any cross-core collective.

Device algorithm (per core, c-major [channel x n] base layout):
  1. LayerNorm stats via replicated ones-matmuls on PE; z = (x-m)*rstd (DVE),
     rstd = exp(-0.5*ln(var+eps)) on ACT (Rsqrt table is unusable here).
  2. EMA as chunked causal convolution on PE, chunk L=128:
     - scale+transpose fused: one matmul per (chunk,dtile,headgroup) with a
       diag(rho_h) packed rhs turns c-major z into n-major per-head scaled
       inputs X_h (4 heads per N=512 matmul).
     - per chunk, 8 lower-triangular T_h matmuls head-accumulate in PSUM,
       plus a K=8 rank-8 carry-correction matmul (q_h^{i+1} profiles).
     - carries tracked per head via an unscaled transpose + end-row matmul
       (E), propagated with tiny [8,512] DVE ops.
  3. Back-transpose to c-major via identity matmuls, residual add on GpSimd,
     DMA out.
"""
import contextlib
import ctypes
import sys
import types

import numpy as np

for _p in ("/root/.axon_site/_ro/trn_rl_repo", "/opt/trn_rl_repo"):
    if _p not in sys.path:
        sys.path.append(_p)

B, C, N, H = 4, 512, 4096, 8
EPS = 1e-5
N_CORES = 8
NHALF = N // 2
CT = C // 128  # channel tiles
L = 128  # EMA chunk length


# ---------------------------------------------------------------------------
# axon NTFF shim (lets run_bass_kernel_spmd(trace=True) capture HW profiles)
# ---------------------------------------------------------------------------
def _install_ntff_shim():
    if "antenv.axon_hooks" in sys.modules:
        return
    holder = {"hook": None}

    def _make(so_path):
        try:
            lib = ctypes.CDLL(so_path)
        except OSError:
            return None
        if not hasattr(lib, "axon_start_nrt_profile"):
            return None
        lib.axon_start_nrt_profile.argtypes = [
            ctypes.POINTER(ctypes.c_int64),
            ctypes.c_size_t,
        ]
        lib.axon_start_nrt_profile.restype = ctypes.c_int64
        lib.axon_stop_nrt_profile.argtypes = [ctypes.c_char_p]
        lib.axon_stop_nrt_profile.restype = ctypes.c_int64

        @contextlib.contextmanager
        def _hook(output_dir, device_ids):
            import jax

            jax.devices()
            if device_ids:
                ids = (ctypes.c_int64 * len(device_ids))(*device_ids)
                rc = lib.axon_start_nrt_profile(ids, len(device_ids))
            else:
                rc = lib.axon_start_nrt_profile(None, 0)
            if rc != 0:
                raise RuntimeError(f"axon_start_nrt_profile rc={rc}")
            try:
                yield
            finally:
                n = lib.axon_stop_nrt_profile(str(output_dir).encode())
                print(f"ntff profile: {n} file(s) -> {output_dir}", file=sys.stderr)

        return _hook

    mod = types.ModuleType("antenv.axon_hooks")
    mod.set_axon_ntff_profile_hook = lambda h: holder.__setitem__("hook", h)
    mod.get_axon_ntff_profile_hook = lambda: holder["hook"]
    sys.modules["antenv.axon_hooks"] = mod
    try:
        import antenv

        antenv.axon_hooks = mod
    except ImportError:
        pass
    holder["hook"] = _make("/opt/axon/libaxon_pjrt.so")


def _split_multiwait(nc, max_waits=1):
    """This walrus build rejects >1 sync wait per instruction; split extras
    onto same-engine NoOps inserted just before (per-engine order is the
    execution order, so semantics are preserved)."""
    from concourse import mybir

    k = [0]
    for fn in nc.m.functions:
        for blk in fn.blocks:
            out = []
            for inst in blk.instructions:
                si = getattr(inst, "sync_info", None)
                if si is not None and len(si.on_wait) > max_waits:
                    waits = list(si.on_wait)
                    for w in waits[max_waits:]:
                        k[0] += 1
                        out.append(
                            mybir.InstNoOp(
                                name=f"{inst.name}-mw{k[0]}",
                                sync_info=mybir.SyncInfo(on_wait=[w], on_update=[]),
                                bass_nofuse=True,
                                engine=inst.engine,
                            )
                        )
                    inst.sync_info = mybir.SyncInfo(
                        on_wait=waits[:max_waits], on_update=list(si.on_update)
                    )
                out.append(inst)
            blk.instructions[:] = out


# ---------------------------------------------------------------------------
# program builder
# ---------------------------------------------------------------------------
def build_program(W):
    """Build the SPMD per-core program. W: halo width (multiple of L)."""
    import concourse.bass as bass
    import concourse.tile as tile
    from concourse import mybir

    NW = NHALF + W
    K0 = W // L
    NCH = NW // L  # chunks
    # ragged 512-wide stat chunks
    stat_slices = []
    o = 0
    while o < NW:
        w = min(512, NW - o)
        stat_slices.append((o, w))
        o += w
    f32 = mybir.dt.float32
    bf16 = mybir.dt.bfloat16
    Op = mybir.AluOpType
    Act = mybir.ActivationFunctionType

    nc = bass.Bass(
        "TRN2",
        target_bir_lowering=False,
        debug=False,
        enable_asserts=False,
        num_devices=N_CORES,
    )
    xs_d = nc.dram_tensor("xs", [C, NW], f32, kind="ExternalInput").ap()
    tm_d = nc.dram_tensor("tmats", [H * 128, 128], bf16, kind="ExternalInput").ap()
    w4_d = nc.dram_tensor("w4", [H * 128, 512], bf16, kind="ExternalInput").ap()
    ek_d = nc.dram_tensor("ek", [128, H], bf16, kind="ExternalInput").ap()
    pm_d = nc.dram_tensor("pmat", [H, 128], bf16, kind="ExternalInput").ap()
    id_d = nc.dram_tensor("ident", [128, 128], bf16, kind="ExternalInput").ap()
    rh_d = nc.dram_tensor("rho_hd", [H, C], f32, kind="ExternalInput").ap()
    ql_d = nc.dram_tensor("qlcol", [H, 1], f32, kind="ExternalInput").ap()
    out_d = nc.dram_tensor("out_t", [C, NHALF], f32, kind="ExternalOutput").ap()

    with tile.TileContext(nc) as tc:
        with contextlib.ExitStack() as ctx:
            pers = ctx.enter_context(tc.tile_pool(name="pers", bufs=1))
            xs_pool = ctx.enter_context(tc.tile_pool(name="xsp", bufs=2))
            sq_pool = ctx.enter_context(tc.tile_pool(name="sqp", bufs=4))
            ps_pool = ctx.enter_context(tc.tile_pool(name="ps", bufs=1, space="PSUM"))
            st_pool = ctx.enter_context(tc.tile_pool(name="stats", bufs=3))
            xh_pool = ctx.enter_context(tc.tile_pool(name="xhp", bufs=4))
            xu_pool = ctx.enter_context(tc.tile_pool(name="xup", bufs=4))
            cr_pool = ctx.enter_context(tc.tile_pool(name="crp", bufs=3))
            s_pool = ctx.enter_context(tc.tile_pool(name="sp", bufs=4))
            out_pool = ctx.enter_context(tc.tile_pool(name="outp", bufs=4))

            # ---- small constants (sync queue, cheap) ----
            ek = pers.tile([128, H], bf16, tag="ek")
            nc.sync.dma_start(out=ek[:], in_=ek_d)
            pmat = pers.tile([H, 128], bf16, tag="pmat")
            nc.sync.dma_start(out=pmat[:], in_=pm_d)
            ident = pers.tile([128, 128], bf16, tag="ident")
            nc.sync.dma_start(out=ident[:], in_=id_d)
            rho = pers.tile([H, C], f32, tag="rho")
            nc.sync.dma_start(out=rho[:], in_=rh_d)
            qlc = pers.tile([H, 1], f32, tag="qlc")
            nc.sync.dma_start(out=qlc[:], in_=ql_d)
            epsb = pers.tile([128, 1], f32, tag="eps")
            nc.gpsimd.memset(epsb[:], EPS)
            ones = pers.tile([128, 128], bf16, tag="ones")
            nc.gpsimd.memset(ones[:], 1.0 / C)
            # big constants on the scalar HWDGE queue so they don't delay xs
            T8 = [pers.tile([128, 128], bf16, tag=f"T{h}", name=f"T{h}") for h in range(H)]
            for h in range(H):
                nc.scalar.dma_start(out=T8[h][:], in_=tm_d[h * 128 : (h + 1) * 128, :])
            W4 = [pers.tile([128, 512], bf16, tag=f"W4_{i}", name=f"W4_{i}") for i in range(H)]
            for i in range(H):
                nc.scalar.dma_start(out=W4[i][:], in_=w4_d[i * 128 : (i + 1) * 128, :])

            # ---- load, cast, square (per stat-chunk pieces for fast ramp) ----
            xb = pers.tile([128, CT * NW], bf16, tag="xb")
            z = pers.tile([128, CT * NW], bf16, tag="z")
            xsq = [pers.tile([128, NW], bf16, tag=f"sq{ct}", name=f"sq{ct}")
                   for ct in range(CT)]
            for o, wd in stat_slices:
                for ct in range(CT):
                    xst = xs_pool.tile([128, 512], f32, tag="xs", bufs=6)
                    nc.sync.dma_start(
                        out=xst[:, :wd],
                        in_=xs_d[ct * 128 : (ct + 1) * 128, o : o + wd],
                    )
                    nc.vector.tensor_scalar(
                        out=xb[:, ct * NW + o : ct * NW + o + wd], in0=xst[:, :wd],
                        scalar1=1.0, scalar2=None, op0=Op.mult,
                    )
                    nc.scalar.square(out=xsq[ct][:, o : o + wd], in_=xst[:, :wd])

            # ---- layernorm stats + z ----
            for o, wd in stat_slices:
                ps_m = ps_pool.tile([128, 512], f32, tag="ema", bufs=2)
                ps_s = ps_pool.tile([128, 512], f32, tag="ema", bufs=2)
                for ct in range(CT):
                    nc.tensor.matmul(
                        out=ps_m[:, :wd], lhsT=ones[:],
                        rhs=xb[:, ct * NW + o : ct * NW + o + wd],
                        start=(ct == 0), stop=(ct == CT - 1),
                    )
                for ct in range(CT):
                    nc.tensor.matmul(
                        out=ps_s[:, :wd], lhsT=ones[:], rhs=xsq[ct][:, o : o + wd],
                        start=(ct == 0), stop=(ct == CT - 1),
                    )
                mean_bf = st_pool.tile([128, 512], bf16, tag="meanbf")
                nc.scalar.activation(out=mean_bf[:, :wd], in_=ps_m[:, :wd], func=Act.Copy)
                m2 = st_pool.tile([128, 512], f32, tag="m2")
                nc.scalar.square(out=m2[:, :wd], in_=ps_m[:, :wd])
                var = st_pool.tile([128, 512], f32, tag="var")
                nc.vector.scalar_tensor_tensor(
                    out=var[:, :wd], in0=ps_s[:, :wd], scalar=0.0, in1=m2[:, :wd],
                    op0=Op.bypass, op1=Op.subtract,
                )
                lnv = st_pool.tile([128, 512], f32, tag="lnv")
                nc.scalar.activation(out=lnv[:, :wd], in_=var[:, :wd], func=Act.Ln, bias=epsb[:])
                rstd = st_pool.tile([128, 512], bf16, tag="rstd")
                nc.scalar.activation(out=rstd[:, :wd], in_=lnv[:, :wd], func=Act.Exp, scale=-0.5)
                for ct in range(CT):
                    t = st_pool.tile([128, 512], bf16, tag="tnorm")
                    nc.vector.tensor_tensor(
                        out=t[:, :wd], in0=xb[:, ct * NW + o : ct * NW + o + wd],
                        in1=mean_bf[:, :wd], op=Op.subtract,
                    )
                    nc.vector.tensor_tensor(
                        out=z[:, ct * NW + o : ct * NW + o + wd], in0=t[:, :wd],
                        in1=rstd[:, :wd], op=Op.mult,
                    )

            # ---- EMA chunks ----
            c_cur = cr_pool.tile([H, C], f32, tag="carry")
            nc.gpsimd.memset(c_cur[:], 0.0)

            def z_slice(k, dt):
                return z[:, dt * NW + k * L : dt * NW + (k + 1) * L]

            def carry_end(k):
                """X_u transpose + end-row matmul E_k; returns e_ps."""
                xu_ps = ps_pool.tile([128, 512], f32, tag="misc", bufs=2)
                for dt in range(CT):
                    nc.tensor.matmul(
                        out=xu_ps[:, dt * 128 : (dt + 1) * 128],
                        lhsT=z_slice(k, dt), rhs=ident[:], start=True, stop=True,
                    )
                xu = xu_pool.tile([128, 512], bf16, tag="xu")
                nc.scalar.activation(out=xu[:], in_=xu_ps[:], func=Act.Copy)
                e_ps = ps_pool.tile([H, 512], f32, tag="misc", bufs=2)
                nc.tensor.matmul(out=e_ps[:], lhsT=ek[:], rhs=xu[:], start=True,
                                 stop=True)
                return e_ps

            def carry_update(c_prev, e_ps):
                c_nxt = cr_pool.tile([H, C], f32, tag="carry")
                c_tmp = cr_pool.tile([H, C], f32, tag="ctmp")
                nc.vector.tensor_scalar(
                    out=c_tmp[:], in0=c_prev[:], scalar1=qlc[:, 0:1], scalar2=None,
                    op0=Op.mult,
                )
                nc.vector.tensor_tensor(out=c_nxt[:], in0=c_tmp[:], in1=e_ps[:],
                                        op=Op.add)
                return c_nxt

            def make_xh(k):
                """scaled transposes: xh cols = g*2048 + dt*512 + h'*128 + jj"""
                xh = xh_pool.tile([128, H * 512], bf16, tag="xh")
                for g in range(2):
                    for dp in range(2):
                        sp = ps_pool.tile([128, 1024], f32, tag="xps", bufs=2)
                        for dd in range(2):
                            dt = dp * 2 + dd
                            nc.tensor.matmul(
                                out=sp[:, dd * 512 : (dd + 1) * 512],
                                lhsT=z_slice(k, dt), rhs=W4[g * CT + dt][:],
                                start=True, stop=True,
                            )
                        dst = xh[:, g * 2048 + dp * 1024 : g * 2048 + (dp + 1) * 1024]
                        if (g + dp) % 2 == 0:
                            nc.scalar.activation(out=dst, in_=sp[:], func=Act.Copy)
                        else:
                            nc.vector.tensor_scalar(
                                out=dst, in0=sp[:], scalar1=1.0, scalar2=None,
                                op0=Op.mult,
                            )
                return xh[:].rearrange("p (g dt hp jj) -> p g dt hp jj",
                                       g=2, dt=CT, hp=4)

            def make_crho(c):
                c_rho = cr_pool.tile([H, C], bf16, tag="crho")
                nc.vector.tensor_tensor(out=c_rho[:], in0=c[:], in1=rho[:], op=Op.mult)
                return c_rho

            def chunk_tail(k, ema_ps):
                """back-transpose (PE identity matmuls) + residual + store"""
                s_sb = s_pool.tile([128, 512], bf16, tag="ssb")
                nc.scalar.activation(out=s_sb[:], in_=ema_ps[:], func=Act.Copy)
                t_ps = ps_pool.tile([128, 512], f32, tag="misc", bufs=2)
                for dt in range(CT):
                    nc.tensor.matmul(
                        out=t_ps[:, dt * 128 : (dt + 1) * 128],
                        lhsT=s_sb[:, dt * 128 : (dt + 1) * 128], rhs=ident[:],
                        start=True, stop=True,
                    )
                o_sb = s_pool.tile([128, 512], bf16, tag="osb")
                nc.scalar.activation(out=o_sb[:], in_=t_ps[:], func=Act.Copy)
                ot = out_pool.tile([128, 512], f32, tag="out")
                resid = xb.rearrange("p (dt t) -> p dt t", dt=CT)[
                    :, :, k * L : (k + 1) * L
                ]
                nc.gpsimd.tensor_tensor(
                    out=ot[:].rearrange("p (dt i) -> p dt i", dt=CT),
                    in0=o_sb[:].rearrange("p (dt i) -> p dt i", dt=CT),
                    in1=resid, op=Op.add,
                )
                ko = k - K0
                nc.sync.dma_start(
                    out=out_d.rearrange("(dt p) n -> p dt n", dt=CT)[
                        :, :, ko * L : (ko + 1) * L
                    ],
                    in_=ot[:].rearrange("p (dt i) -> p dt i", dt=CT),
                )

            for k in range(K0):  # halo chunks: carries only
                e_ps = carry_end(k)
                c_cur = carry_update(c_cur, e_ps)

            ks = list(range(K0, NCH))
            pairs = [ks[i : i + 2] for i in range(0, len(ks), 2)]
            for pair in pairs:
                xhs, crhos, psums = [], [], []
                for k in pair:
                    last = k == NCH - 1
                    e_ps = None if last else carry_end(k)
                    xhs.append(make_xh(k))
                    crhos.append(make_crho(c_cur))
                    if not last:
                        c_cur = carry_update(c_cur, e_ps)
                for h in range(H):  # interleave pair to reuse T8[h] stationary
                    g, hp = divmod(h, 4)
                    for i, k in enumerate(pair):
                        if h == 0:
                            psums.append(ps_pool.tile([128, 512], f32, tag="ema",
                                                      bufs=2, name=f"emaps{k}"))
                        nc.tensor.matmul(
                            out=psums[i][:], lhsT=T8[h][:], rhs=xhs[i][:, g, :, hp, :],
                            start=(h == 0), stop=False,
                        )
                for i, k in enumerate(pair):
                    nc.tensor.matmul(
                        out=psums[i][:], lhsT=pmat[:], rhs=crhos[i][:], start=False,
                        stop=True,
                    )
                for i, k in enumerate(pair):
                    chunk_tail(k, psums[i])
    return nc


def _host_params(ln_gamma, ln_beta, expansion, reduction, alphas, dampen_factors):
    import ml_dtypes

    a = 1.0 / (1.0 + np.exp(-alphas.astype(np.float64)))
    q = (1.0 - a) / (1.0 + np.exp(-dampen_factors.astype(np.float64)))
    qmax = float(q.max())
    W = L
    while qmax**W > 1e-12 and W < NHALF:
        W += L
    rho = (
        a[:, None]
        * expansion.astype(np.float64)
        * reduction.astype(np.float64)
        * ln_gamma.astype(np.float64)[None, :]
    )  # [H, C]
    bf = ml_dtypes.bfloat16
    ii, jj = np.meshgrid(np.arange(L), np.arange(L), indexing="ij")
    tmats = np.zeros((H * 128, 128), bf)
    for h in range(H):
        M = np.where(ii >= jj, q[h] ** np.maximum(ii - jj, 0), 0.0)  # T_h[i,j]
        tmats[h * 128 : (h + 1) * 128, :] = M.T.astype(bf)  # lhsT[j,i]
    w4 = np.zeros((H * 128, 512), bf)
    for g in range(2):
        for dt in range(CT):
            blk = np.zeros((128, 512))
            for hp in range(4):
                h = g * 4 + hp
                blk[:, hp * 128 : (hp + 1) * 128] = np.diag(rho[h, dt * 128 : (dt + 1) * 128])
            w4[(g * CT + dt) * 128 : (g * CT + dt + 1) * 128, :] = blk.astype(bf)
    ek = np.zeros((128, H), bf)
    for h in range(H):
        ek[:, h] = (q[h] ** (L - 1 - np.arange(L))).astype(bf)
    pmat = np.zeros((H, 128), bf)
    for h in range(H):
        pmat[h, :] = (q[h] ** (np.arange(L) + 1.0)).astype(bf)
    ident = np.eye(128, dtype=bf)
    rho_hd = rho.astype(np.float32)
    qlcol = (q**L).astype(np.float32).reshape(H, 1)
    consts = dict(
        tmats=tmats, w4=w4, ek=ek, pmat=pmat, ident=ident, rho_hd=rho_hd,
        qlcol=qlcol,
    )
    return a, q, W, consts


def _beta_term(ln_beta, expansion, reduction, a, q):
    if not np.any(ln_beta):
        return None
    n_idx = np.arange(N, dtype=np.float64)
    Cn = a[:, None] * (1.0 - q[:, None] ** (n_idx[None, :] + 1.0)) / (1.0 - q[:, None])
    w = (
        expansion.astype(np.float64)
        * reduction.astype(np.float64)
        * ln_beta.astype(np.float64)[None, :]
    )
    return np.einsum("hc,hn->cn", w, Cn).astype(np.float32)


def _make_in_maps(x, W, consts):
    NW = NHALF + W
    in_maps = []
    for core in range(N_CORES):
        b, half = divmod(core, 2)
        xs = np.zeros((C, NW), np.float32)
        s = half * NHALF - W
        if s < 0:
            xs[:, W:] = x[b, :, :NHALF]
        else:
            xs[:] = x[b, :, s : s + NW]
        in_maps.append(dict(consts, xs=xs))
    return in_maps


def kernel(x, ln_gamma, ln_beta, expansion, reduction, alphas, dampen_factors,
           trace=False):
    _install_ntff_shim()
    from concourse.bass_utils import run_bass_kernel_spmd
    from concourse.bass_interp import get_hw_module

    x = np.asarray(x, np.float32)
    a, q, W, consts = _host_params(
        np.asarray(ln_gamma), np.asarray(ln_beta), np.asarray(expansion),
        np.asarray(reduction), np.asarray(alphas), np.asarray(dampen_factors),
    )
    nc = build_program(W)
    _split_multiwait(nc)
    nc.m = get_hw_module(nc.m)

    in_maps = _make_in_maps(x, W, consts)
    res = run_bass_kernel_spmd(
        nc, in_maps, core_ids=list(range(N_CORES)), trace=trace
    )

    out = np.empty((B, C, N), np.float32)
    for core in range(N_CORES):
        b, half = divmod(core, 2)
        out[b, :, half * NHALF : (half + 1) * NHALF] = res.results[core]["out_t"]
    bt = _beta_term(
        np.asarray(ln_beta), np.asarray(expansion), np.asarray(reduction), a, q
    )
    if bt is not None:
        out += bt[None]
    if trace:
        kernel.last_results = res
    return out
